# revision 26
# baseline (speedup 1.0000x reference)
"""LSTMCell on 8 Trainium2 NeuronCores, data-parallel over the batch.

Full inputs: x/h_t/c_t [65536,128] f32, 8 gate weight matrices [128,128],
4 biases [128]. Returns (h_new, c_new) as [65536,128] f32 each.

fp16 end-to-end on device (tolerance 2e-2; fp16 keeps ~2e-3), halving
HBM traffic vs f32, "gatesT" layout (host pre-transposes everything so
no on-chip transposes are needed), ACT-bound steady state:
  - Per 512-col batch group: 8 fp16 matmuls accumulate a 4-bank PSUM
    quad [i|f|o|s], s=sigmoid(2g) slot (W_g pre-scaled by 2 so
    tanh(g)=2s-1); ONE sigmoid per quad on ACT; DVE fp16 2x ops; tanh
    per block on ACT; h = o * tanh(c).
  - HWDGE transfers serialize per ring with ~2us completion latency
    each, so the host packs ALL inputs into ONE staging tensor laid out
    in transfer order ([w][x0 h0][c0 x1 h1 c1][g2-3]...): few transfers,
    each one sem, ping-ponged across the sync and ACT HWDGE rings
    (ACT-ring triggers are all pre-stream, before the first sigmoid).
  - Outputs hn|cn pack into one tensor; one 3D-AP DMA per block with
    descending block sizes; the last tiny block rides the ACT ring
    after the sigmoid stream has ended.
"""
import numpy as np
from contextlib import ExitStack

import concourse.bass as bass
import concourse.tile as tile
from concourse import bacc, mybir
from concourse.bass_utils import run_bass_kernel_spmd

F32 = mybir.dt.float32
F16 = mybir.dt.float16
AF = mybir.ActivationFunctionType
ALU = mybir.AluOpType

NCORES = 8
BC = 8192            # batch rows per core
GW = 512             # batch cols per group
NG = BC // GW        # 16 groups

# Staging segments in transfer order: (ring, [items]) where an item is
# 'w' or (kind, group). Group bundle = x,h,c each [128, 512] fp16.
SEGS = [
    ("scalar", ["w"]),
    ("sync",   [("x", 0), ("h", 0), ("x", 1), ("h", 1)]),
    ("scalar", [("c", 0), ("c", 1), ("x", 2), ("h", 2), ("c", 2)]),
    ("sync",   [(k, g) for g in (3, 4) for k in "xhc"]),
    ("scalar", [(k, g) for g in (5, 6) for k in "xhc"]),
    ("sync",   [(k, g) for g in (7, 8, 9) for k in "xhc"]),
    ("scalar", [(k, g) for g in (10, 11, 12) for k in "xhc"]),
    ("sync",   [(k, g) for g in (13, 14, 15) for k in "xhc"]),
]
SEG_COLS = [sum(1024 if it == "w" else 512 for it in items)
            for _, items in SEGS]
SEG_OFF = np.cumsum([0] + SEG_COLS).tolist()
INP_COLS = SEG_OFF[-1]

# group -> (segment idx, x off, h off, c off) within the segment
GMAP = {}
for si, (_, items) in enumerate(SEGS):
    off = 0
    for it in items:
        if it == "w":
            off += 1024
            continue
        k, g = it
        GMAP.setdefault(g, [si, None, None, None])
        assert GMAP[g][0] == si or k == "c"
        GMAP[g]["xhc".index(k) + 1] = (si, off)
        off += 512

# output blocks == tanh chunks: each block's packed hn|cn DMA is issued
# as soon as its tanh chunk + muls complete, keeping the output ring
# drained; tanh chunks are staggered so the PE never hits the 2-quad
# PSUM wall. Tail blocks ride the ACT HWDGE ring (idle post-stream).
BLOCKS = [[0, 1, 2, 3], [4, 5, 6, 7], [8, 9, 10, 11], [12, 13], [14], [15]]
TCHUNKS = BLOCKS
TPOS = {5: 0, 8: 1, 12: 2, 14: 3, 15: 4}   # after sigmoid g -> tanh chunk
BPOS = {5: 0, 9: 1, 13: 2}                 # after group g DVE -> finish blk

_CACHE = {}


def _build(has_bias: bool):
    nc = bacc.Bacc("TRN2", target_bir_lowering=False, debug=False)
    inp = nc.dram_tensor("inp", [128, INP_COLS], F16,
                         kind="ExternalInput").ap()
    if has_bias:
        bias = nc.dram_tensor("bias", [1, 512], F16, kind="ExternalInput").ap()
    out = nc.dram_tensor("out", [128, 2 * BC], F16, kind="ExternalOutput").ap()
    out3 = out.rearrange("p (k c) -> p k c", k=2)

    g2b = {g: b for b, gs in enumerate(BLOCKS) for g in gs}

    with tile.TileContext(nc) as tc:
        with ExitStack() as ctx:
            const = ctx.enter_context(tc.tile_pool(name="const", bufs=1))
            ip = ctx.enter_context(tc.tile_pool(name="ip", bufs=len(SEGS)))
            qp = ctx.enter_context(tc.tile_pool(name="qp", bufs=2, space="PSUM"))
            sp = ctx.enter_context(tc.tile_pool(name="sp", bufs=11))
            tmp = ctx.enter_context(tc.tile_pool(name="tmp", bufs=4))
            obp = ctx.enter_context(tc.tile_pool(name="obp", bufs=4))
            tcp = ctx.enter_context(tc.tile_pool(name="tcp", bufs=2))

            z = const.tile([128, 8], F16)
            nc.vector.memset(z[:], 0.0)
            if has_bias:
                ones = const.tile([1, GW], F16)
                nc.vector.memset(ones[:], 1.0)
                b_sb = const.tile([1, 512], F16)
                nc.sync.dma_start(b_sb[:], bias)

            # input segment transfers: one DMA each, both HWDGE rings;
            # all ACT-ring triggers are pre-stream.
            segt = []
            for si, (ring, _) in enumerate(SEGS):
                t = ip.tile([128, SEG_COLS[si]], F16, name=f"seg{si}",
                            tag="seg")
                segt.append(t)
                eng = nc.scalar if ring == "scalar" else nc.sync
                eng.dma_start(t[:], inp[:, SEG_OFF[si]:SEG_OFF[si + 1]])

            w_sb = segt[0]  # [128, 1024] = [WxT | WhT]
            wx_sb = w_sb[:, 0:512]
            wh_sb = w_sb[:, 512:1024]

            # ACT table warm-up (after the ACT-ring triggers)
            zs = const.tile([128, 8], F16)
            nc.scalar.activation(zs[:], z[:], AF.Sigmoid)

            # PE p-state warm-up: enough back-to-back ops (~25ns apiece)
            # to keep the PE continuously busy until the weight DMA
            # lands, then a few wide ops on the weights bridge the rest
            # of the wait, so the real matmuls run at the ramped clock
            # instead of the cold ~1.6x-slower one.
            warm = qp.tile([128, 2048], F32, name="warm", tag="quad")
            for _ in range(200):
                nc.tensor.matmul(warm[0:8, 0:8], z[:], z[:],
                                 start=True, stop=True)

            sigs = [None] * NG
            obs = [None] * len(BLOCKS)
            tcs = [None] * len(TCHUNKS)
            g2blk = {g: b for b, gs in enumerate(BLOCKS) for g in gs}

            def gslice(g, kind):
                si, off = GMAP[g]["xhc".index(kind) + 1]
                return segt[si][:, off:off + GW]

            def tanh_chunk(c):
                """tanh over chunk c of c_new (a slice of its block's ob)."""
                groups = TCHUNKS[c]
                b = g2blk[groups[0]]
                bn = len(BLOCKS[b]) * GW
                j0 = BLOCKS[b].index(groups[0])
                n = len(groups) * GW
                tcs[c] = tcp.tile([128, n], F16, name=f"tc{c}", tag="tc")
                nc.scalar.activation(
                    tcs[c][:], obs[b][:, bn + j0 * GW:bn + j0 * GW + n],
                    AF.Tanh)

            def finish_block(b, on_scalar=False):
                """h = o*tanh(c) muls + packed hn|cn output DMA."""
                groups = BLOCKS[b]
                lo = groups[0] * GW
                n = len(groups) * GW
                ob = obs[b]
                for g in groups:
                    c = next(ci for ci, gs in enumerate(TCHUNKS) if g in gs)
                    coff = (g - TCHUNKS[c][0]) * GW
                    gsl = slice((g - groups[0]) * GW, (g - groups[0] + 1) * GW)
                    nc.vector.tensor_mul(ob[:, gsl],
                                         sigs[g][:, 1024:1536],
                                         tcs[c][:, coff:coff + GW])
                eng = nc.scalar if on_scalar else nc.sync
                eng.dma_start(out3[:, :, lo:lo + n],
                              ob[:].rearrange("p (k c) -> p k c", k=2))

            for g in range(NG):
                b = g2b[g]
                j = BLOCKS[b].index(g)
                xg = gslice(g, "x")
                hg = gslice(g, "h")
                cg = gslice(g, "c")

                quad = qp.tile([128, 2048], F32, name=f"q{g}", tag="quad")
                for blk in range(4):
                    osl = slice(blk * GW, (blk + 1) * GW)
                    wsl = slice(blk * 128, (blk + 1) * 128)
                    first = True
                    if has_bias:
                        nc.tensor.matmul(quad[:, osl], b_sb[:, wsl],
                                         ones[:], start=True, stop=False)
                        first = False
                    nc.tensor.matmul(quad[:, osl], wx_sb[:, wsl], xg,
                                     start=first, stop=False)
                    nc.tensor.matmul(quad[:, osl], wh_sb[:, wsl], hg,
                                     start=False, stop=True)

                sig = sp.tile([128, 2048], F16, name=f"s{g}", tag="sig")
                sigs[g] = sig
                nc.scalar.activation(sig[:], quad[:], AF.Sigmoid)

                # staggered tanh chunk right after this sigmoid
                if g in TPOS:
                    tanh_chunk(TPOS[g])

                gt = tmp.tile([128, GW], F16, name=f"gt{g}", tag="t")
                nc.vector.tensor_scalar(gt[:], sig[:, 1536:2048], 2.0, 1.0,
                                        ALU.mult, ALU.subtract)
                ig = tmp.tile([128, GW], F16, name=f"ig{g}", tag="t")
                nc.vector.tensor_mul(ig[:], sig[:, 0:512], gt[:])
                fc = tmp.tile([128, GW], F16, name=f"fc{g}", tag="t")
                nc.vector.tensor_mul(fc[:], sig[:, 512:1024], cg)
                if j == 0:
                    bn = len(BLOCKS[b]) * GW
                    obs[b] = obp.tile([128, 2 * bn], F16, name=f"ob{b}",
                                      tag="ob")
                bn = len(BLOCKS[b]) * GW
                nc.vector.tensor_add(
                    obs[b][:, bn + j * GW:bn + (j + 1) * GW], ig[:], fc[:])
                if g in BPOS:
                    finish_block(BPOS[g])

            # tail: [12,13] and [14] flush via the ACT ring (idle after
            # the stream), [15] via the sync ring (free after block 2).
            finish_block(3, on_scalar=True)
            finish_block(4, on_scalar=True)
            tanh_chunk(5)
            finish_block(5)
    nc.compile()
    return nc


def _run(inputs, trace=False, tmpdir=None):
    # gate order [i, f, o, g] with W_g/b_g doubled: tanh(g)=2*sigmoid(2g)-1
    wx = np.concatenate([inputs["W_ii"], inputs["W_if"], inputs["W_io"],
                         2.0 * np.asarray(inputs["W_ig"])], axis=0)
    wh = np.concatenate([inputs["W_hi"], inputs["W_hf"], inputs["W_ho"],
                         2.0 * np.asarray(inputs["W_hg"])], axis=0)
    b = np.concatenate([inputs["b_i"], inputs["b_f"], inputs["b_o"],
                        2.0 * np.asarray(inputs["b_g"])], axis=0)
    w = np.ascontiguousarray(
        np.concatenate([np.asarray(wx).T, np.asarray(wh).T], axis=1),
        dtype=np.float16)
    has_bias = bool(np.any(b))

    xt = np.asarray(inputs["x"], dtype=np.float16).T    # [128, 65536]
    ht_ = np.asarray(inputs["h_t"], dtype=np.float16).T
    ct_ = np.asarray(inputs["c_t"], dtype=np.float16).T
    tens = {"x": xt, "h": ht_, "c": ct_}

    key = has_bias
    if key not in _CACHE:
        _CACHE[key] = _build(has_bias)
    nc = _CACHE[key]

    in_maps = []
    for i in range(NCORES):
        base = i * BC
        inp = np.empty((128, INP_COLS), dtype=np.float16)
        for si, (_, items) in enumerate(SEGS):
            off = SEG_OFF[si]
            for it in items:
                if it == "w":
                    inp[:, off:off + 1024] = w
                    off += 1024
                else:
                    k, g = it
                    lo = base + g * GW
                    inp[:, off:off + GW] = tens[k][:, lo:lo + GW]
                    off += GW
        m = {"inp": inp}
        if has_bias:
            m["bias"] = b.reshape(1, 512).astype(np.float16)
        in_maps.append(m)

    res = run_bass_kernel_spmd(nc, in_maps, core_ids=list(range(NCORES)),
                               trace=trace, tmpdir=tmpdir)
    h_new = np.empty((NCORES * BC, 128), dtype=np.float32)
    c_new = np.empty((NCORES * BC, 128), dtype=np.float32)
    for i, r in enumerate(res.results):
        sl = slice(i * BC, (i + 1) * BC)
        o = np.asarray(r["out"], dtype=np.float32)
        h_new[sl] = o[:, :BC].T
        c_new[sl] = o[:, BC:].T
    return h_new, c_new, res


def kernel(**inputs):
    h_new, c_new, _ = _run(inputs, trace=False)
    return h_new, c_new


# revision 27
# speedup vs baseline: 1.0156x; 1.0156x over previous
"""LSTMCell on 8 Trainium2 NeuronCores, data-parallel over the batch.

Full inputs: x/h_t/c_t [65536,128] f32, 8 gate weight matrices [128,128],
4 biases [128]. Returns (h_new, c_new) as [65536,128] f32 each.

fp16 end-to-end on device (tolerance 2e-2; fp16 keeps ~2e-3), halving
HBM traffic vs f32, "gatesT" layout (host pre-transposes everything so
no on-chip transposes are needed), ACT-bound steady state:
  - Per 512-col batch group: 8 fp16 matmuls accumulate a 4-bank PSUM
    quad [i|f|o|s], s=sigmoid(2g) slot (W_g pre-scaled by 2 so
    tanh(g)=2s-1); ONE sigmoid per quad on ACT; DVE fp16 2x ops; tanh
    per block on ACT; h = o * tanh(c).
  - HWDGE transfers serialize per ring with ~2us completion latency
    each, so the host packs ALL inputs into ONE staging tensor laid out
    in transfer order ([w][x0 h0][c0 x1 h1 c1][g2-3]...): few transfers,
    each one sem, ping-ponged across the sync and ACT HWDGE rings
    (ACT-ring triggers are all pre-stream, before the first sigmoid).
  - Outputs hn|cn pack into one tensor; one 3D-AP DMA per block with
    descending block sizes; the last tiny block rides the ACT ring
    after the sigmoid stream has ended.
"""
import numpy as np
from contextlib import ExitStack

import concourse.bass as bass
import concourse.tile as tile
from concourse import bacc, mybir
from concourse.bass_utils import run_bass_kernel_spmd

F32 = mybir.dt.float32
F16 = mybir.dt.float16
AF = mybir.ActivationFunctionType
ALU = mybir.AluOpType

NCORES = 8
BC = 8192            # batch rows per core
GW = 512             # batch cols per group
NG = BC // GW        # 16 groups

# Staging segments in transfer order: (ring, [items]) where an item is
# 'w' or (kind, group). Group bundle = x,h,c each [128, 512] fp16.
SEGS = [
    ("scalar", ["w"]),
    ("sync",   [("x", 0), ("h", 0), ("x", 1), ("h", 1)]),
    ("scalar", [("c", 0), ("c", 1), ("x", 2), ("h", 2), ("c", 2)]),
    ("sync",   [(k, g) for g in (3, 4) for k in "xhc"]),
    ("scalar", [(k, g) for g in (5, 6) for k in "xhc"]),
    ("sync",   [(k, g) for g in (7, 8, 9) for k in "xhc"]),
    ("scalar", [(k, g) for g in (10, 11, 12) for k in "xhc"]),
    ("sync",   [(k, g) for g in (13, 14, 15) for k in "xhc"]),
]
SEG_COLS = [sum(1024 if it == "w" else 512 for it in items)
            for _, items in SEGS]
SEG_OFF = np.cumsum([0] + SEG_COLS).tolist()
INP_COLS = SEG_OFF[-1]

# group -> (segment idx, x off, h off, c off) within the segment
GMAP = {}
for si, (_, items) in enumerate(SEGS):
    off = 0
    for it in items:
        if it == "w":
            off += 1024
            continue
        k, g = it
        GMAP.setdefault(g, [si, None, None, None])
        assert GMAP[g][0] == si or k == "c"
        GMAP[g]["xhc".index(k) + 1] = (si, off)
        off += 512

# output blocks == tanh chunks: each block's packed hn|cn DMA is issued
# as soon as its tanh chunk + muls complete, keeping the output ring
# drained; tanh chunks are staggered so the PE never hits the 2-quad
# PSUM wall. Tail blocks ride the ACT HWDGE ring (idle post-stream).
BLOCKS = [[0, 1, 2, 3], [4, 5, 6, 7], [8, 9, 10, 11], [12, 13], [14], [15]]
TCHUNKS = BLOCKS
TPOS = {5: 0, 8: 1, 12: 2, 14: 3, 15: 4}   # after sigmoid g -> tanh chunk
BPOS = {5: 0, 9: 1, 13: 2}                 # after group g DVE -> finish blk

_CACHE = {}


def _build(has_bias: bool):
    nc = bacc.Bacc("TRN2", target_bir_lowering=False, debug=False)
    inp = nc.dram_tensor("inp", [128, INP_COLS], F16,
                         kind="ExternalInput").ap()
    if has_bias:
        bias = nc.dram_tensor("bias", [1, 512], F16, kind="ExternalInput").ap()
    out = nc.dram_tensor("out", [128, 2 * BC], F16, kind="ExternalOutput").ap()
    out3 = out.rearrange("p (k c) -> p k c", k=2)

    g2b = {g: b for b, gs in enumerate(BLOCKS) for g in gs}

    with tile.TileContext(nc) as tc:
        with ExitStack() as ctx:
            const = ctx.enter_context(tc.tile_pool(name="const", bufs=1))
            ip = ctx.enter_context(tc.tile_pool(name="ip", bufs=len(SEGS)))
            qp = ctx.enter_context(tc.tile_pool(name="qp", bufs=2, space="PSUM"))
            sp = ctx.enter_context(tc.tile_pool(name="sp", bufs=11))
            tmp = ctx.enter_context(tc.tile_pool(name="tmp", bufs=4))
            obp = ctx.enter_context(tc.tile_pool(name="obp", bufs=4))
            tcp = ctx.enter_context(tc.tile_pool(name="tcp", bufs=2))

            z = const.tile([128, 8], F16)
            nc.vector.memset(z[:], 0.0)
            if has_bias:
                ones = const.tile([1, GW], F16)
                nc.vector.memset(ones[:], 1.0)
                b_sb = const.tile([1, 512], F16)
                nc.sync.dma_start(b_sb[:], bias)

            # input segment transfers: one DMA each, both HWDGE rings;
            # all ACT-ring triggers are pre-stream.
            segt = []
            for si, (ring, _) in enumerate(SEGS):
                t = ip.tile([128, SEG_COLS[si]], F16, name=f"seg{si}",
                            tag="seg")
                segt.append(t)
                eng = nc.scalar if ring == "scalar" else nc.sync
                eng.dma_start(t[:], inp[:, SEG_OFF[si]:SEG_OFF[si + 1]])

            w_sb = segt[0]  # [128, 1024] = [WxT | WhT]
            wx_sb = w_sb[:, 0:512]
            wh_sb = w_sb[:, 512:1024]

            # ACT table warm-up (after the ACT-ring triggers)
            zs = const.tile([128, 8], F16)
            nc.scalar.activation(zs[:], z[:], AF.Sigmoid)

            # PE pipeline warm-up (the first wide matmuls after an idle
            # PE run ~1.6x slow regardless; more warmups than this just
            # overrun into the real work).
            warm = qp.tile([128, 2048], F32, name="warm", tag="quad")
            for _ in range(24):
                nc.tensor.matmul(warm[0:8, 0:8], z[:], z[:],
                                 start=True, stop=True)

            sigs = [None] * NG
            obs = [None] * len(BLOCKS)
            tcs = [None] * len(TCHUNKS)
            g2blk = {g: b for b, gs in enumerate(BLOCKS) for g in gs}

            def gslice(g, kind):
                si, off = GMAP[g]["xhc".index(kind) + 1]
                return segt[si][:, off:off + GW]

            def tanh_chunk(c):
                """tanh over chunk c of c_new (a slice of its block's ob)."""
                groups = TCHUNKS[c]
                b = g2blk[groups[0]]
                bn = len(BLOCKS[b]) * GW
                j0 = BLOCKS[b].index(groups[0])
                n = len(groups) * GW
                tcs[c] = tcp.tile([128, n], F16, name=f"tc{c}", tag="tc")
                nc.scalar.activation(
                    tcs[c][:], obs[b][:, bn + j0 * GW:bn + j0 * GW + n],
                    AF.Tanh)

            def finish_block(b, on_scalar=False):
                """h = o*tanh(c) muls + packed hn|cn output DMA."""
                groups = BLOCKS[b]
                lo = groups[0] * GW
                n = len(groups) * GW
                ob = obs[b]
                for g in groups:
                    c = next(ci for ci, gs in enumerate(TCHUNKS) if g in gs)
                    coff = (g - TCHUNKS[c][0]) * GW
                    gsl = slice((g - groups[0]) * GW, (g - groups[0] + 1) * GW)
                    nc.vector.tensor_mul(ob[:, gsl],
                                         sigs[g][:, 1024:1536],
                                         tcs[c][:, coff:coff + GW])
                eng = nc.scalar if on_scalar else nc.sync
                eng.dma_start(out3[:, :, lo:lo + n],
                              ob[:].rearrange("p (k c) -> p k c", k=2))

            for g in range(NG):
                b = g2b[g]
                j = BLOCKS[b].index(g)
                xg = gslice(g, "x")
                hg = gslice(g, "h")
                cg = gslice(g, "c")

                quad = qp.tile([128, 2048], F32, name=f"q{g}", tag="quad")
                for blk in range(4):
                    osl = slice(blk * GW, (blk + 1) * GW)
                    wsl = slice(blk * 128, (blk + 1) * 128)
                    first = True
                    if has_bias:
                        nc.tensor.matmul(quad[:, osl], b_sb[:, wsl],
                                         ones[:], start=True, stop=False)
                        first = False
                    nc.tensor.matmul(quad[:, osl], wx_sb[:, wsl], xg,
                                     start=first, stop=False)
                    nc.tensor.matmul(quad[:, osl], wh_sb[:, wsl], hg,
                                     start=False, stop=True)

                sig = sp.tile([128, 2048], F16, name=f"s{g}", tag="sig")
                sigs[g] = sig
                nc.scalar.activation(sig[:], quad[:], AF.Sigmoid)

                # staggered tanh chunk right after this sigmoid
                if g in TPOS:
                    tanh_chunk(TPOS[g])

                gt = tmp.tile([128, GW], F16, name=f"gt{g}", tag="t")
                nc.vector.tensor_scalar(gt[:], sig[:, 1536:2048], 2.0, 1.0,
                                        ALU.mult, ALU.subtract)
                ig = tmp.tile([128, GW], F16, name=f"ig{g}", tag="t")
                nc.vector.tensor_mul(ig[:], sig[:, 0:512], gt[:])
                fc = tmp.tile([128, GW], F16, name=f"fc{g}", tag="t")
                nc.vector.tensor_mul(fc[:], sig[:, 512:1024], cg)
                if j == 0:
                    bn = len(BLOCKS[b]) * GW
                    obs[b] = obp.tile([128, 2 * bn], F16, name=f"ob{b}",
                                      tag="ob")
                bn = len(BLOCKS[b]) * GW
                nc.vector.tensor_add(
                    obs[b][:, bn + j * GW:bn + (j + 1) * GW], ig[:], fc[:])
                if g in BPOS:
                    finish_block(BPOS[g])

            # tail: [12,13] and [14] flush via the ACT ring (idle after
            # the stream), [15] via the sync ring (free after block 2).
            finish_block(3, on_scalar=True)
            finish_block(4, on_scalar=True)
            tanh_chunk(5)
            finish_block(5)
    nc.compile()
    return nc


def _run(inputs, trace=False, tmpdir=None):
    # gate order [i, f, o, g] with W_g/b_g doubled: tanh(g)=2*sigmoid(2g)-1
    wx = np.concatenate([inputs["W_ii"], inputs["W_if"], inputs["W_io"],
                         2.0 * np.asarray(inputs["W_ig"])], axis=0)
    wh = np.concatenate([inputs["W_hi"], inputs["W_hf"], inputs["W_ho"],
                         2.0 * np.asarray(inputs["W_hg"])], axis=0)
    b = np.concatenate([inputs["b_i"], inputs["b_f"], inputs["b_o"],
                        2.0 * np.asarray(inputs["b_g"])], axis=0)
    w = np.ascontiguousarray(
        np.concatenate([np.asarray(wx).T, np.asarray(wh).T], axis=1),
        dtype=np.float16)
    has_bias = bool(np.any(b))

    xt = np.asarray(inputs["x"], dtype=np.float16).T    # [128, 65536]
    ht_ = np.asarray(inputs["h_t"], dtype=np.float16).T
    ct_ = np.asarray(inputs["c_t"], dtype=np.float16).T
    tens = {"x": xt, "h": ht_, "c": ct_}

    key = has_bias
    if key not in _CACHE:
        _CACHE[key] = _build(has_bias)
    nc = _CACHE[key]

    in_maps = []
    for i in range(NCORES):
        base = i * BC
        inp = np.empty((128, INP_COLS), dtype=np.float16)
        for si, (_, items) in enumerate(SEGS):
            off = SEG_OFF[si]
            for it in items:
                if it == "w":
                    inp[:, off:off + 1024] = w
                    off += 1024
                else:
                    k, g = it
                    lo = base + g * GW
                    inp[:, off:off + GW] = tens[k][:, lo:lo + GW]
                    off += GW
        m = {"inp": inp}
        if has_bias:
            m["bias"] = b.reshape(1, 512).astype(np.float16)
        in_maps.append(m)

    res = run_bass_kernel_spmd(nc, in_maps, core_ids=list(range(NCORES)),
                               trace=trace, tmpdir=tmpdir)
    h_new = np.empty((NCORES * BC, 128), dtype=np.float32)
    c_new = np.empty((NCORES * BC, 128), dtype=np.float32)
    for i, r in enumerate(res.results):
        sl = slice(i * BC, (i + 1) * BC)
        o = np.asarray(r["out"], dtype=np.float32)
        h_new[sl] = o[:, :BC].T
        c_new[sl] = o[:, BC:].T
    return h_new, c_new, res


def kernel(**inputs):
    h_new, c_new, _ = _run(inputs, trace=False)
    return h_new, c_new
